# revision 1
# baseline (speedup 1.0000x reference)
"""CQAttention (BiDAF context-query attention) forward kernel for 8 Trainium2
NeuronCores.

Full inputs: context (64,128,1024) f32, question (64,128,128) f32, w (384,) f32.
Full output: (64, 512, 1024) f32.

Sharding: pure data parallel over batch — 8 batches per core, w replicated.

Math (per batch, X = context[b] (H,C), Y = question[b] (H,Q), w=(wq,wc,wcq)):
    S^T = (wcq*Y + wc 1^T)^T @ X              # (Q,C); wq term is softmax-invariant
    P   = exp(S^T)                            # unnormalized softmax numerators
    d   = rowsum(P); r = 1/d                  # softmax denominators (per q-row)
    A   = (diag(r) Y^T)^T @ P                 # = a^T                (H,C)
    tt  = P @ X^T                             # (Q,H) via PE transposes of P,X
    Bm  = (diag(r^2) tt)^T @ P                # = b^T = (s1 (s1^T c))^T  (H,C)
    out = [X; A; X*A; X*Bm]                   # (4H, C)

All matmuls run in float32r (TF32-like, ~1 cycle/row at N>=256 vs 4 for fp32).
X/Y are DMA'd directly into f32r-typed tiles (bit passthrough) so no rounding
pass is needed; engines touching the data elementwise use .bitcast(f32) views.
"""

import os
import sys

import numpy as np

if "/opt/trn_rl_repo" not in sys.path:
    sys.path.insert(0, "/opt/trn_rl_repo")

B, H, C, Q = 64, 128, 1024, 128
NCORES = 8
BPC = B // NCORES  # batches per core


def _ensure_ntff_hook():
    """This container's `antenv` stub lacks `axon_hooks`, which
    bass_utils needs for NTFF profiling under axon (trace=True). Install
    a functional shadow module + register the ctypes-based hook."""
    import types

    try:
        from antenv.axon_hooks import get_axon_ntff_profile_hook  # noqa: F401

        return  # real module present
    except ImportError:
        pass
    try:
        import antenv

        mod = types.ModuleType("antenv.axon_hooks")
        _state = {"hook": None}

        def set_axon_ntff_profile_hook(h):
            _state["hook"] = h

        def get_axon_ntff_profile_hook():
            return _state["hook"]

        mod.set_axon_ntff_profile_hook = set_axon_ntff_profile_hook
        mod.get_axon_ntff_profile_hook = get_axon_ntff_profile_hook
        sys.modules["antenv.axon_hooks"] = mod
        antenv.axon_hooks = mod

        from trn_agent_boot.trn_boot import _ntff_profile_via_ctypes

        set_axon_ntff_profile_hook(
            _ntff_profile_via_ctypes("/opt/axon/libaxon_pjrt.so")
        )
    except Exception:
        pass  # profiling degrades; compute still works


_ensure_ntff_hook()

LAST_RESULTS = None
_NC = None


def _build():
    from contextlib import ExitStack

    import concourse.bacc as bacc
    import concourse.mybir as mybir
    import concourse.tile as tile
    from concourse import masks

    f32 = mybir.dt.float32
    f32r = mybir.dt.float32r
    EXP = mybir.ActivationFunctionType.Exp
    IDENT = mybir.ActivationFunctionType.Identity

    nc = bacc.Bacc(
        "TRN2", target_bir_lowering=False, debug=False, enable_asserts=False
    )
    ctx_t = nc.dram_tensor("context", (BPC, H, C), f32, kind="ExternalInput").ap()
    q_t = nc.dram_tensor("question", (BPC, H, Q), f32, kind="ExternalInput").ap()
    w_t = nc.dram_tensor("w", (3 * H,), f32, kind="ExternalInput").ap()
    # device writes only blocks 1..3 (A, X*A, X*B); block 0 == context is
    # filled host-side during unshard (pure passthrough of an input).
    out_t = nc.dram_tensor("out", (BPC, 3 * H, C), f32, kind="ExternalOutput").ap()

    with tile.TileContext(nc) as tc, ExitStack() as ctx:
        const = ctx.enter_context(tc.tile_pool(name="const", bufs=1))
        sb = ctx.enter_context(tc.tile_pool(name="sb", bufs=3))
        sbx = ctx.enter_context(tc.tile_pool(name="sbx", bufs=3))
        # PSUM: one unified half-bank pool (6 slots) + yt/tt pool (2 slots)
        ps_tr = ctx.enter_context(tc.tile_pool(name="ptr", bufs=6, space="PSUM"))
        ps_tt = ctx.enter_context(tc.tile_pool(name="ptt", bufs=2, space="PSUM"))

        ident = const.tile([128, 128], f32, tag="ident")
        masks.make_identity(nc, ident[:])
        identr = const.tile([128, 128], f32r, tag="identr")
        nc.vector.tensor_copy(identr[:], ident[:])

        # w arrives as one contiguous (1,384) row (cheap single-descriptor
        # DMA); the (128,1) columns are produced by K=1 PE matmuls against
        # identity — avoids two slow 128-descriptor scatter DMAs at startup.
        w_row = const.tile([1, 3 * H], f32r, tag="w_row")
        nc.sync.dma_start(w_row[:], w_t.unsqueeze(0).bitcast(f32r))
        wc = const.tile([128, 1], f32, tag="wc")
        wcq = const.tile([128, 1], f32, tag="wcq")

        # Software-pipelined emission with a 1-batch skew: each engine's
        # stream is in-order, so batch b's tail (B matmuls, output copies,
        # out-DMAs) is emitted inside batch b+1's front to fill the
        # exp/tts latency gaps on the PE stream.
        state = {}  # live tiles of the in-flight batch

        def front(b):
            # X keeps exact f32 bits (for X*A / X*B); Xr/Yr are round-to-
            # nearest f32r copies for the matmul/transpose paths (the PE
            # truncates raw f32 bits, which costs ~2x the rounding error).
            Y = sb.tile([H, Q], f32, tag="Y")
            nc.sync.dma_start(Y[:], q_t[b])
            X = sbx.tile([H, C], f32, tag="X")
            if b == 0:
                nc.sync.dma_start(X[:, 0:512], ctx_t[b, :, 0:512])
                nc.sync.dma_start(X[:, 512:1024], ctx_t[b, :, 512:1024])
            else:
                nc.sync.dma_start(X[:], ctx_t[b])
            Xr = sbx.tile([H, C], f32r, tag="Xr")
            nc.vector.tensor_copy(Xr[:, 0:512], X[:, 0:512])
            nc.vector.tensor_copy(Xr[:, 512:1024], X[:, 512:1024])
            Yr = sb.tile([H, Q], f32r, tag="Yr")
            nc.vector.tensor_copy(Yr[:], Y[:])
            if b == 0:
                wps = ps_tr.tile([128, 512], f32, tag="tr")
                nc.tensor.matmul(
                    wps[:, 0:128],
                    w_row[0:1, H : 2 * H],
                    identr[0:1, 0:128],
                    start=True,
                    stop=True,
                )
                nc.tensor.matmul(
                    wps[:, 128:256],
                    w_row[0:1, 2 * H : 3 * H],
                    identr[0:1, 0:128],
                    start=True,
                    stop=True,
                )
                nc.vector.tensor_copy(wc[:], wps[:, 0:1])
                nc.vector.tensor_copy(wcq[:], wps[:, 128:129])
            # Z = wcq * Y + wc  (so Z^T @ X = G + 1 cw^T, the softmax logits).
            # On ACT so batch b+1's scores never wait behind DVE's copy burst.
            Z = sb.tile([H, Q], f32r, tag="Z")
            nc.scalar.activation(
                Z[:], Y[:], IDENT, bias=wc[:], scale=wcq[:]
            )

            # scores + exp, in two 512-wide halves for pipelining
            P = sb.tile([Q, C], f32r, tag="P")
            dh = sb.tile([Q, 2], f32, tag="dh")
            yt = ps_tt.tile([128, 256], f32, tag="tt")
            for j in range(2):
                Sh = ps_tr.tile([Q, 512], f32, tag="tr")
                nc.tensor.matmul(
                    Sh[:], Z[:], Xr[:, j * 512 : (j + 1) * 512], start=True, stop=True
                )
                nc.scalar.activation(
                    P[:, j * 512 : (j + 1) * 512],
                    Sh[:],
                    EXP,
                    accum_out=dh[:, j : j + 1],
                )
            nc.tensor.transpose(yt[:, 0:128].bitcast(f32r), Yr[:], identr[:])
            dsum = sb.tile([Q, 1], f32, tag="dsum")
            nc.vector.tensor_add(dsum[:], dh[:, 0:1], dh[:, 1:2])
            rr = sb.tile([Q, 1], f32, tag="rr")
            nc.vector.reciprocal(rr[:], dsum[:])
            r2 = sb.tile([Q, 1], f32, tag="r2")
            nc.vector.tensor_mul(r2[:], rr[:], rr[:])

            # XT holds [YTs | X^T chunks 0..7]; the leading YTs block means
            # every N=256 tt-matmul window reads initialized data.
            XT = sb.tile([128, 128 + C], f32r, tag="XT")
            YTs = XT[:, 0:128]
            nc.vector.tensor_scalar_mul(YTs, yt[:, 0:128], rr[:])

            state.update(X=X, Xr=Xr, P=P, rr=rr, r2=r2, XT=XT, b=b)

        def mid(b):
            X, P, XT, r2 = state["Xr"], state["P"], state["XT"], state["r2"]
            YTs = XT[:, 0:128]
            # X^T chunks first (independent of exp), then P^T chunks
            for g in range(2):
                xtp = ps_tr.tile([128, 512], f32, tag="tr")
                for k in range(4):
                    c0 = g * 4 + k
                    nc.tensor.transpose(
                        xtp[:, k * 128 : (k + 1) * 128].bitcast(f32r),
                        X[:, c0 * 128 : (c0 + 1) * 128],
                        identr[:],
                    )
                nc.scalar.copy(XT[:, 128 + g * 512 : 128 + (g + 1) * 512], xtp[:])

            PT = sb.tile([128, C], f32r, tag="PT")
            for g in range(2):
                ptp = ps_tr.tile([128, 512], f32, tag="tr")
                for k in range(4):
                    c0 = g * 4 + k
                    nc.tensor.transpose(
                        ptp[:, k * 128 : (k + 1) * 128].bitcast(f32r),
                        P[:, c0 * 128 : (c0 + 1) * 128],
                        identr[:],
                    )
                nc.scalar.copy(PT[:, g * 512 : (g + 1) * 512], ptp[:])

            # tt[:,128:256] = P @ X^T  (cols 0:128 accumulate junk, never read)
            tt = ps_tt.tile([Q, 256], f32, tag="tt")
            for c in range(8):
                nc.tensor.matmul(
                    tt[:],
                    PT[:, c * 128 : (c + 1) * 128],
                    XT[:, c * 128 : c * 128 + 256],
                    start=(c == 0),
                    stop=(c == 7),
                )
            tts = sb.tile([Q, H], f32r, tag="tts")
            nc.vector.tensor_scalar_mul(tts[:], tt[:, 128:256], r2[:])

            # A into AB[:, 0:1024]
            AB = sb.tile([H, 2 * C], f32, tag="AB")
            for j in range(2):
                Aps = ps_tr.tile([H, 512], f32, tag="tr")
                nc.tensor.matmul(
                    Aps[:], YTs, P[:, j * 512 : (j + 1) * 512], start=True, stop=True
                )
                nc.vector.tensor_copy(AB[:, j * 512 : (j + 1) * 512], Aps[:])
            state.update(tts=tts, AB=AB)

        def back(b):
            X, P, tts, AB = state["X"], state["P"], state["tts"], state["AB"]
            Xf = X[:]
            for j in range(2):
                Bps = ps_tr.tile([H, 512], f32, tag="tr")
                nc.tensor.matmul(
                    Bps[:], tts[:], P[:, j * 512 : (j + 1) * 512], start=True, stop=True
                )
                if j == 0:
                    nc.scalar.copy(AB[:, C : C + 512], Bps[:])
                else:
                    nc.vector.tensor_copy(AB[:, C + 512 : 2 * C], Bps[:])

            # XA / XB as separate DVE ops so each output DMA fires as soon
            # as its half is ready; output DMAs spread over sync + gpsimd.
            # The final batch is chunked finer so the closing transfers
            # overlap the compute tail.
            last = b == BPC - 1
            XAB = sb.tile([H, 2 * C], f32, tag="XAB")
            step = 512 if last else C
            for o in range(0, C, step):
                nc.vector.tensor_mul(
                    XAB[:, o : o + step], Xf[:, o : o + step], AB[:, o : o + step]
                )
                nc.sync.dma_start(
                    out_t[b, H : 2 * H, o : o + step], XAB[:, o : o + step]
                )
            (nc.sync if last else nc.gpsimd).dma_start(
                out_t[b, 0:H], AB[:, 0:C]
            )
            for o in range(C, 2 * C, step):
                nc.vector.tensor_mul(
                    XAB[:, o : o + step],
                    Xf[:, o - C : o - C + step],
                    AB[:, o : o + step],
                )
                nc.sync.dma_start(
                    out_t[b, 2 * H : 3 * H, o - C : o - C + step], XAB[:, o : o + step]
                )

        prev = None
        for b in range(BPC):
            front(b)
            if prev is not None:
                back_state = prev
                # back(b-1) was deferred: restore its tiles, emit, restore b's
                cur = dict(state)
                state.clear()
                state.update(back_state)
                back(b - 1)
                state.clear()
                state.update(cur)
            mid(b)
            prev = dict(state)
        back(BPC - 1)

    nc.compile()
    return nc


def kernel(context, question, w):
    global _NC, LAST_RESULTS
    from concourse import bass_utils

    if _NC is None:
        _NC = _build()

    context = np.ascontiguousarray(np.asarray(context), dtype=np.float32)
    question = np.ascontiguousarray(np.asarray(question), dtype=np.float32)
    w = np.ascontiguousarray(np.asarray(w), dtype=np.float32)

    in_maps = [
        {
            "context": context[c * BPC : (c + 1) * BPC],
            "question": question[c * BPC : (c + 1) * BPC],
            "w": w,
        }
        for c in range(NCORES)
    ]
    trace = bool(int(os.environ.get("KTRACE", "0")))
    LAST_RESULTS = bass_utils.run_bass_kernel_spmd(
        _NC, in_maps, core_ids=list(range(NCORES)), trace=trace
    )
    out = np.empty((B, 4 * H, C), dtype=np.float32)
    out[:, 0:H, :] = context
    for c in range(NCORES):
        out[c * BPC : (c + 1) * BPC, H:, :] = LAST_RESULTS.results[c]["out"]
    return out

